# revision 1
# baseline (speedup 1.0000x reference)
"""LoRA Linear kernel for Trainium2, 8 NeuronCores.

Computes out = x @ (W + lora_A @ lora_B)^T + bias for
x [4, 2048, 4096], W [4096, 4096], lora_A [4096, 16], lora_B [16, 4096].

Sharding: 2-way over tokens (M = 8192 -> 4096/core) x 4-way over
out_features (4096 -> 1024/core). Host pre-transposes/pre-tiles x and W
so every DMA is a large contiguous 2D transfer with the contraction dim
(in_features) on partitions; the LoRA delta is folded into W^T on-device
with rank-16 matmuls (Wtot^T = W^T + B^T A^T), interleaved with the
first three token tiles' accumulation so the PE never head-of-line
blocks on the W stream. Operands are bf16 (fp32 PSUM accumulation) —
the chip-level HBM envelope (~175 GB/s/core with all 8 cores pulling)
makes the f32 variant memory-bound, while bf16 is PE-bound.
"""

import ml_dtypes

import numpy as np

import concourse.bass as bass
import concourse.bacc as bacc
import concourse.mybir as mybir
import concourse.tile as tile
from concourse.bass_utils import run_bass_kernel_spmd

IN_F = 4096
OUT_F = 4096
RANK = 16
BATCH, SEQ = 4, 2048
M_TOT = BATCH * SEQ          # 8192 tokens
MG, OG = 2, 4                # shard grid: token-groups x outfeature-groups
M_LOC = M_TOT // MG          # 4096 tokens per core
O_LOC = OUT_F // OG          # 1024 out features per core
P = 128
KI = IN_F // P               # 32 contraction tiles
NF = 512                     # matmul moving free dim (one PSUM bank)
OS = O_LOC // NF             # 2 output column passes
MT = M_LOC // P              # 32 token tiles per core

F32 = mybir.dt.float32
BF16 = mybir.dt.bfloat16

_cache = {}


def _build():
    nc = bacc.Bacc(None, target_bir_lowering=False)

    # x pre-tiled on host to [MT, P, KI, P]: (mt, i_within, i_tile, m)
    xt = nc.dram_tensor("xt", [MT, P, KI, P], BF16, kind="ExternalInput")
    wt = nc.dram_tensor("wt", [IN_F, O_LOC], BF16, kind="ExternalInput")
    lb = nc.dram_tensor("lb", [RANK, IN_F], F32, kind="ExternalInput")
    at = nc.dram_tensor("at", [RANK, O_LOC], F32, kind="ExternalInput")
    br = nc.dram_tensor("br", [P, O_LOC], F32, kind="ExternalInput")
    out = nc.dram_tensor("out", [M_LOC, O_LOC], F32, kind="ExternalOutput")

    with tile.TileContext(nc) as tc:
        with (
            tc.tile_pool(name="const", bufs=1) as const_pool,
            tc.tile_pool(name="wfold", bufs=3) as wfold_pool,
            tc.tile_pool(name="xin", bufs=4) as xin_pool,
            tc.tile_pool(name="outs", bufs=3) as out_pool,
            tc.tile_pool(name="psum", bufs=2, space="PSUM") as psum_pool,
            tc.tile_pool(name="psum_mm", bufs=3, space="PSUM") as psum_mm_pool,
        ):
            # resident folded weight, [i_within, i_tile, o] = W^T + B^T A^T
            wtot = const_pool.tile([P, KI, O_LOC], BF16, name="wtot")
            a_raw = const_pool.tile([RANK, O_LOC], F32, name="a_raw")
            a_sb = const_pool.tile([RANK, O_LOC], BF16, name="a_sb")
            bias_sb = const_pool.tile([P, O_LOC], F32, name="bias_sb")
            nc.gpsimd.dma_start(a_raw[:], at[:])
            nc.vector.tensor_copy(out=a_sb[:], in_=a_raw[:])
            nc.gpsimd.dma_start(bias_sb[:], br[:])

            def load_x(mt):
                x_tile = xin_pool.tile([P, KI, P], BF16, name="x_tile", tag="x_tile")
                eng = nc.sync if mt % 2 == 0 else nc.gpsimd
                eng.dma_start(x_tile[:], xt[mt])
                return x_tile

            def mm_pair(x_tile, ki, psums):
                for os_ in range(OS):
                    nc.tensor.matmul(
                        psums[os_][:],
                        x_tile[:, ki, :],
                        wtot[:, ki, os_ * NF : (os_ + 1) * NF],
                        start=(ki == 0),
                        stop=(ki == KI - 1),
                    )

            def store_out(mt, psums):
                for os_ in range(OS):
                    o_tile = out_pool.tile([P, NF], F32, name="o_tile", tag="o_tile")
                    nc.vector.tensor_add(
                        out=o_tile[:],
                        in0=psums[os_][:],
                        in1=bias_sb[:, os_ * NF : (os_ + 1) * NF],
                    )
                    nc.scalar.dma_start(
                        out[mt * P : (mt + 1) * P, os_ * NF : (os_ + 1) * NF],
                        o_tile[:],
                    )

            # ---- m_tiles 0..2, interleaved with the W fold ----
            # The W fold streams 16 MiB; striping it over 3 DMA queues and
            # overlapping three token tiles' matmuls keeps the PE busy while
            # it lands.
            NLEAD = 3
            wt_engines = [nc.gpsimd, nc.scalar, nc.sync]
            lead_x = [load_x(mt) for mt in range(NLEAD)]
            lead_psums = [
                [
                    psum_mm_pool.tile(
                        [P, NF], F32, name=f"psum_{mt}_{os_}", tag=f"ps{os_}"
                    )
                    for os_ in range(OS)
                ]
                for mt in range(NLEAD)
            ]
            for ki in range(KI):
                wt_tile = wfold_pool.tile([P, O_LOC], BF16, name="wt_tile")
                wt_engines[ki % 3].dma_start(wt_tile[:], wt[ki * P : (ki + 1) * P, :])
                b_raw = wfold_pool.tile([RANK, P], F32, name="b_raw", bufs=2)
                b_sb = wfold_pool.tile([RANK, P], BF16, name="b_sb", bufs=2)
                nc.sync.dma_start(b_raw[:], lb[:, ki * P : (ki + 1) * P])
                nc.vector.tensor_copy(out=b_sb[:], in_=b_raw[:])
                for os_ in range(OS):
                    dpsum = psum_pool.tile([P, NF], F32, name="dpsum", tag="dpsum")
                    nc.tensor.matmul(
                        dpsum[:],
                        b_sb[:],
                        a_sb[:, os_ * NF : (os_ + 1) * NF],
                        start=True,
                        stop=True,
                    )
                    nc.vector.tensor_add(
                        out=wtot[:, ki, os_ * NF : (os_ + 1) * NF],
                        in0=dpsum[:],
                        in1=wt_tile[:, os_ * NF : (os_ + 1) * NF],
                    )
                for mt in range(NLEAD):
                    mm_pair(lead_x[mt], ki, lead_psums[mt])
            for mt in range(NLEAD):
                store_out(mt, lead_psums[mt])

            # ---- m_tiles NLEAD..MT-1 ----
            for mt in range(NLEAD, MT):
                x_tile = load_x(mt)
                psums = [
                    psum_mm_pool.tile([P, NF], F32, name=f"psum{os_}", tag=f"ps{os_}")
                    for os_ in range(OS)
                ]
                for ki in range(KI):
                    mm_pair(x_tile, ki, psums)
                store_out(mt, psums)
    nc.finalize()
    return nc


def kernel(x, W, bias, lora_A, lora_B):
    x = np.asarray(x, dtype=np.float32)
    W = np.asarray(W, dtype=np.float32)
    bias = np.asarray(bias, dtype=np.float32)
    lora_A = np.asarray(lora_A, dtype=np.float32)
    lora_B = np.asarray(lora_B, dtype=np.float32)

    if "nc" not in _cache:
        _cache["nc"] = _build()
    nc = _cache["nc"]

    xr = x.reshape(M_TOT, IN_F).astype(ml_dtypes.bfloat16)
    in_maps = []
    for c in range(8):
        mg, og = c % MG, c // MG
        xs = xr[mg * M_LOC : (mg + 1) * M_LOC]
        # [M_LOC, IN_F] -> (mt, m, ki, p) -> (mt, p, ki, m)
        xs = np.ascontiguousarray(xs.reshape(MT, P, KI, P).transpose(0, 3, 2, 1))
        in_maps.append(
            {
                "xt": xs,
                "wt": np.ascontiguousarray(W[og * O_LOC : (og + 1) * O_LOC].T.astype(ml_dtypes.bfloat16)),
                "lb": np.ascontiguousarray(lora_B),
                "at": np.ascontiguousarray(lora_A[og * O_LOC : (og + 1) * O_LOC].T),
                "br": np.ascontiguousarray(
                    np.broadcast_to(bias[og * O_LOC : (og + 1) * O_LOC], (P, O_LOC))
                ),
            }
        )

    res = run_bass_kernel_spmd(nc, in_maps, core_ids=list(range(8)))

    out = np.empty((M_TOT, OUT_F), dtype=np.float32)
    for c in range(8):
        mg, og = c % MG, c // MG
        out[mg * M_LOC : (mg + 1) * M_LOC, og * O_LOC : (og + 1) * O_LOC] = res.results[
            c
        ]["out"]
    return out.reshape(BATCH, SEQ, OUT_F)

